# revision 17
# baseline (speedup 1.0000x reference)
"""Distributed Trainium2 kernel for nn_AttentionBlock (channel attention).

Algorithm (exact algebra, no approximation):
  The attention matrix is [C,C] with the contraction over N=H*W*D tokens.
  GroupNorm is a per-channel affine xn = a*x + b whose stats derive from
  per-channel sums s = x@1 and the Gram matrix G = x@x.T (diag(G) = sumsq).
  Everything downstream of G is [C,C]-sized:
      S    = Wq' G Wk'^T + rank-1 terms        (Wq' = Wq diag(a))
      attn = softmax(S/sqrt(C))
      out  = x + P attn Wv' x + delta 1^T
  So the kernel does: pass 1 (G + s, reduced over local N-shard), a ~1MB
  AllReduce over the 4 cores sharing a batch, a small on-chip [512,512]
  chain, and pass 2 (one [C,C]x[C,N] matmul + residual).

Matmuls use float32r (reduced-precision fp32 at full PE rate; measured
~1.5e-4 per-matmul error). The BIR verifier requires every f32r-matmul
input to be produced by an instruction whose output dtype is float32r,
so tiles on f32r paths are f32r-typed and loads are rounded in place.

Sharding: batch B=2 x sequence 4  ->  8 cores. replica groups [[0..3],[4..7]].
"""

from contextlib import ExitStack

import numpy as np

import concourse.bass as bass
import concourse.tile as tile
from concourse import bacc, mybir
from concourse.bass_utils import run_bass_kernel_spmd
from concourse.masks import make_identity

# Problem constants (hardcoded per harness contract)
B = 2
C = 512
N = 32768          # 32*32*32
NCORES = 8
SHARDS = 4         # sequence shards per batch
NS = N // SHARDS   # 8192 per-core tokens
GROUPS = 32
GSIZE = C // GROUPS  # 16
EPS = 1e-5
P = 128
CT = C // P        # 4 channel tiles
F32 = mybir.dt.float32
F32R = mybir.dt.float32r

PH1_CHUNK = 128
PH1_ITERS = NS // PH1_CHUNK     # 64
PH2_CHUNK = 512
PH2_ITERS = NS // PH2_CHUNK     # 16

REPLICA_GROUPS = [[0, 1, 2, 3], [4, 5, 6, 7]]
SCALE = 1.0 / float(np.sqrt(C))


def f32_(ap):
    return ap.bitcast(F32)


def r_(ap):
    return ap.bitcast(F32R)


def build_graph():
    nc = bacc.Bacc(
        "TRN2", target_bir_lowering=False, debug=False, num_devices=NCORES
    )

    x_ext = nc.dram_tensor("x", [C, NS], F32, kind="ExternalInput")
    gn_w_ext = nc.dram_tensor("gn_w", [C], F32, kind="ExternalInput")
    gn_b_ext = nc.dram_tensor("gn_b", [C], F32, kind="ExternalInput")
    qkv_w_ext = nc.dram_tensor("qkv_w", [3 * C, C], F32, kind="ExternalInput")
    qkv_b_ext = nc.dram_tensor("qkv_b", [3 * C], F32, kind="ExternalInput")
    proj_w_ext = nc.dram_tensor("proj_w", [C, C], F32, kind="ExternalInput")
    proj_b_ext = nc.dram_tensor("proj_b", [C], F32, kind="ExternalInput")
    adjc_ext = nc.dram_tensor("adjc", [P, P], F32, kind="ExternalInput")
    out_ext = nc.dram_tensor("out", [C, NS], F32, kind="ExternalOutput")

    with tile.TileContext(nc) as tc:
        _body(tc, x_ext, gn_w_ext, gn_b_ext, qkv_w_ext, qkv_b_ext,
              proj_w_ext, proj_b_ext, adjc_ext, out_ext)

    nc.compile()
    return nc


def _body(tc, x_ext, gn_w_ext, gn_b_ext, qkv_w_ext, qkv_b_ext,
          proj_w_ext, proj_b_ext, adjc_ext, out_ext):
    nc = tc.nc
    AX = mybir.AxisListType
    OP = mybir.AluOpType
    ACTF = mybir.ActivationFunctionType

    x_view = x_ext[:].rearrange("(ct p) n -> p ct n", p=P)        # [128,4,NS]
    out_view = out_ext[:].rearrange("(ct p) n -> p ct n", p=P)

    ctx = ExitStack()
    consts = ctx.enter_context(tc.tile_pool(name="consts", bufs=1))
    small = ctx.enter_context(tc.tile_pool(name="small", bufs=1))
    wpool = ctx.enter_context(tc.tile_pool(name="wpool", bufs=1))
    xres_pool = ctx.enter_context(tc.tile_pool(name="xres", bufs=1))
    xt_pool = ctx.enter_context(tc.tile_pool(name="xt", bufs=2))
    xstage_pool = ctx.enter_context(tc.tile_pool(name="xstage", bufs=2))
    chain = ctx.enter_context(tc.tile_pool(name="chain", bufs=3))
    gb_pool = ctx.enter_context(tc.tile_pool(name="gbp", bufs=2))
    y_pool = ctx.enter_context(tc.tile_pool(name="yp", bufs=2))
    ps_g = ctx.enter_context(tc.tile_pool(name="psg", bufs=4, space="PSUM"))
    ps_t = ctx.enter_context(tc.tile_pool(name="pst", bufs=2, space="PSUM"))
    ps_y = ctx.enter_context(tc.tile_pool(name="psy", bufs=2, space="PSUM"))
    dram = ctx.enter_context(tc.tile_pool(name="dram", bufs=1, space="DRAM"))

    # ---------------- constants ----------------
    ident = consts.tile([P, P], F32, name="ident")
    make_identity(nc, ident)
    ident_r = consts.tile([P, P], F32R, name="ident_r")
    nc.vector.tensor_copy(ident_r, ident)

    adj = consts.tile([P, P], F32, name="adj")          # 16x16 block-diag ones
    nc.sync.dma_start(adj, adjc_ext[:])

    gw_sb = consts.tile([P, CT], F32, name="gw_sb")
    gb_sb = consts.tile([P, CT], F32, name="gb_sb")
    pb_sb = consts.tile([P, CT], F32, name="pb_sb")
    nc.sync.dma_start(gw_sb, gn_w_ext[:].rearrange("(t p) -> p t", p=P))
    nc.sync.dma_start(gb_sb, gn_b_ext[:].rearrange("(t p) -> p t", p=P))
    nc.sync.dma_start(pb_sb, proj_b_ext[:].rearrange("(t p) -> p t", p=P))
    qkvb_sb = consts.tile([P, 3 * CT], F32, name="qkvb_sb")
    nc.sync.dma_start(qkvb_sb, qkv_b_ext[:].rearrange("(t p) -> p t", p=P))

    # ---------------- weight transposes (pre-AR, independent of x) --------
    # WqT / WkT: [c_in partition, ct, o free]; PwT: [m partition, mt, o free]
    WqT = wpool.tile([P, CT, C], F32R, name="WqT")
    WkT = wpool.tile([P, CT, C], F32R, name="WkT")
    PwT = wpool.tile([P, CT, C], F32R, name="PwT")

    for Wdst, src_ap, label in (
        (WqT, qkv_w_ext[0:C, :], "wq"),
        (WkT, qkv_w_ext[C:2 * C, :], "wk"),
        (PwT, proj_w_ext[:], "pw"),
    ):
        w_stage = chain.tile([P, CT, C], F32, name=f"stage_{label}", tag="c8")
        nc.sync.dma_start(w_stage, src_ap.rearrange("(j p) c -> p j c", p=P))
        for ct in range(CT):
            wps = ps_t.tile([P, C], F32, name=f"wps_{label}_{ct}", tag="pt")
            for j in range(CT):
                nc.tensor.transpose(
                    wps[:, j * P:(j + 1) * P],
                    w_stage[:, j, ct * P:(ct + 1) * P],
                    ident,
                )
            nc.scalar.copy(Wdst[:, ct, :], wps)   # cast -> f32r (rounds)

    # ---------------- phase 1: G = x x^T, s = x @ 1 ----------------
    x_res = xres_pool.tile([P, CT, NS], F32R, name="x_res")
    s_acc = consts.tile([P, PH1_ITERS, CT], F32, name="s_acc")

    G_ps = [ps_g.tile([P, C], F32, name=f"G_ps{ct}", tag="g") for ct in range(CT)]

    for u in range(PH1_ITERS):
        sl = slice(u * PH1_CHUNK, (u + 1) * PH1_CHUNK)
        xs = x_res[:, :, sl]
        xstage = xstage_pool.tile([P, CT, PH1_CHUNK], F32,
                                  name=f"xstage{u}", tag="xs")
        nc.sync.dma_start(xstage, x_view[:, :, sl])
        # rounding copy f32 -> f32r (verifier: f32r inputs need f32r producer)
        nc.vector.tensor_copy(xs, xstage)
        xt_ps = ps_t.tile([P, C], F32, name=f"xt_ps{u}", tag="pt")
        for ct in range(CT):
            nc.tensor.transpose(r_(xt_ps[:, ct * P:(ct + 1) * P]),
                                xs[:, ct, :], ident_r)
        xt = xt_pool.tile([P, C], F32R, name=f"xt{u}", tag="xt")
        nc.scalar.copy(xt, xt_ps)
        for ct in range(CT):
            nc.tensor.matmul(
                G_ps[ct],
                xt[:, ct * P:(ct + 1) * P],
                xt[:],
                start=(u == 0),
                stop=(u == PH1_ITERS - 1),
            )
        nc.vector.reduce_sum(s_acc[:, u, :], xstage, axis=AX.X)

    s_sb = small.tile([P, CT], F32, name="s_sb")
    nc.vector.reduce_sum(
        s_sb, s_acc[:].rearrange("p u t -> p t u"), axis=AX.X
    )

    # ---------------- AllReduce of [G | s] ----------------
    CC = C * C
    cc_in = dram.tile([CC + C], F32, name="cc_in")
    cc_out = dram.tile([CC + C], F32, name="cc_out")

    G_sb = chain.tile([P, CT, C], F32, name="G_sb", tag="c8")
    for ct in range(CT):
        nc.scalar.copy(G_sb[:, ct, :], G_ps[ct])
    nc.sync.dma_start(
        cc_in[0:CC].rearrange("(ct p d) -> p ct d", p=P, d=C), G_sb
    )
    nc.sync.dma_start(
        cc_in[CC:CC + C].rearrange("(p t) -> p t", t=CT), s_sb
    )

    nc.gpsimd.collective_compute(
        "AllReduce",
        OP.add,
        ins=[cc_in[:]],
        outs=[cc_out[:]],
        replica_groups=REPLICA_GROUPS,
    )

    sbar = small.tile([P, CT], F32, name="sbar")
    nc.sync.dma_start(sbar, cc_out[CC:CC + C].rearrange("(p t) -> p t", t=CT))
    diag = small.tile([P, CT], F32, name="diag")
    # diagonal of Gbar: element (p, t) at flat offset p*(C+1) + t*(C*P + P)
    diag_src = bass.AP(
        tensor=cc_out.tensor,
        offset=cc_out.offset,
        ap=[[C + 1, P], [C * P + P, CT]],
    )
    nc.sync.dma_start(diag, diag_src)

    # ---------------- stats -> a, bvec ----------------
    sd_stack = small.tile([P, CT, 2], F32, name="sd_stack")
    nc.vector.tensor_copy(sd_stack[:, :, 0], sbar)
    nc.vector.tensor_copy(sd_stack[:, :, 1], diag)

    gsd = small.tile([P, CT, 2], F32, name="gsd")
    for ct in range(CT):
        gsd_ps = ps_t.tile([P, 2], F32, name=f"gsd_ps{ct}", tag="pt")
        nc.tensor.matmul(gsd_ps, adj, sd_stack[:, ct, :], start=True, stop=True)
        nc.vector.tensor_copy(gsd[:, ct, :], gsd_ps)

    invN = 1.0 / float(GSIZE * N)
    meanex2 = small.tile([P, CT, 2], F32, name="meanex2")
    nc.vector.tensor_scalar_mul(meanex2, gsd, invN)
    mean = meanex2[:, :, 0]
    ex2 = meanex2[:, :, 1]
    msq = small.tile([P, CT], F32, name="msq")
    nc.vector.tensor_mul(out=msq, in0=mean, in1=mean)
    var = small.tile([P, CT], F32, name="var")
    # var + eps = (ex2 + eps) - mean^2
    nc.vector.scalar_tensor_tensor(
        out=var, in0=ex2, scalar=EPS, in1=msq, op0=OP.add, op1=OP.subtract
    )
    sd_ = small.tile([P, CT], F32, name="sd_")
    nc.scalar.sqrt(sd_, var)
    rstd = small.tile([P, CT], F32, name="rstd")
    nc.vector.reciprocal(rstd, sd_)
    a_sb = small.tile([P, CT], F32, name="a_sb")
    nc.vector.tensor_mul(out=a_sb, in0=rstd, in1=gw_sb)
    ma = small.tile([P, CT], F32, name="ma")
    nc.vector.tensor_mul(out=ma, in0=mean, in1=a_sb)
    bvec = small.tile([P, CT], F32, name="bvec")
    nc.vector.tensor_tensor(out=bvec, in0=gb_sb, in1=ma, op=OP.subtract)
    u1 = small.tile([P, CT], F32, name="u1")
    nc.vector.tensor_mul(out=u1, in0=a_sb, in1=sbar)

    uv2 = small.tile([P, CT, 2], F32, name="uv2")
    nc.vector.tensor_copy(uv2[:, :, 0], u1)
    nc.vector.tensor_copy(uv2[:, :, 1], bvec)

    # ---------------- tq/bq, tk/bk (use UNscaled WqT/WkT) ----------------
    # tb[:, j, 0] = W(a*s); tb[:, j, 1] = W bvec (+ qkv bias)
    tb_q = small.tile([P, CT, 2], F32, name="tb_q")
    tb_k = small.tile([P, CT, 2], F32, name="tb_k")
    for tb, WT, bias_off in ((tb_q, WqT, 0), (tb_k, WkT, CT)):
        for j in range(CT):
            tb_ps = ps_t.tile([P, 2], F32, name=f"tb_ps{bias_off}_{j}", tag="pt")
            for ct in range(CT):
                nc.tensor.matmul(
                    tb_ps,
                    f32_(WT[:, ct, j * P:(j + 1) * P]),
                    uv2[:, ct, :],
                    start=(ct == 0),
                    stop=(ct == CT - 1),
                )
            nc.vector.tensor_copy(tb[:, j, :], tb_ps)
            nc.vector.tensor_add(
                out=tb[:, j, 1:2],
                in0=tb[:, j, 1:2],
                in1=qkvb_sb[:, bias_off + j:bias_off + j + 1],
            )

    # scale WqT/WkT in place by a (per input-channel partition)
    for WT in (WqT, WkT):
        for ct in range(CT):
            nc.vector.tensor_scalar_mul(
                WT[:, ct, :], f32_(WT[:, ct, :]), a_sb[:, ct:ct + 1]
            )

    # wk2 = tk + N*bk
    wk2 = small.tile([P, CT], F32, name="wk2")
    nc.vector.tensor_scalar(
        wk2, tb_k[:, :, 1], float(N), None, OP.mult
    )
    nc.vector.tensor_add(out=wk2, in0=wk2, in1=tb_k[:, :, 0])

    # ---------------- rank-1 padded operands ----------------
    # Lpad[0,j,:]=tq(j-slice), Lpad[1,j,:]=bq ; Rpad[0,:]=bk^T, Rpad[1,:]=wk2^T
    Lpad = consts.tile([P, CT, P], F32, name="Lpad")
    Rpad = consts.tile([P, C], F32, name="Rpad")
    nc.gpsimd.memset(Lpad, 0.0)
    nc.gpsimd.memset(Rpad, 0.0)

    rstack = small.tile([P, CT, 2], F32, name="rstack")
    nc.vector.tensor_copy(rstack[:, :, 0], tb_k[:, :, 1])
    nc.vector.tensor_copy(rstack[:, :, 1], wk2)

    for j in range(CT):
        lt_ps = ps_t.tile([2, P], F32, name=f"lt_ps{j}", tag="pt")
        nc.tensor.transpose(lt_ps, tb_q[:, j, :], ident)
        nc.vector.tensor_copy(Lpad[0:2, j, :], lt_ps)
        rt_ps = ps_t.tile([2, P], F32, name=f"rt_ps{j}", tag="pt")
        nc.tensor.transpose(rt_ps, rstack[:, j, :], ident)
        nc.vector.tensor_copy(Rpad[0:2, j * P:(j + 1) * P], rt_ps)

    # ---------------- V = G @ WkT_a   [c part, ok free] ----------------
    V_ps = [ps_g.tile([P, C], F32, name=f"V_ps{j}", tag="g") for j in range(CT)]
    for dt in range(CT):
        gb_stage = gb_pool.tile([P, C], F32, name=f"gbs{dt}", tag="gb")
        nc.sync.dma_start(
            gb_stage,
            cc_out[0:CC].rearrange("(ct p d) -> p ct d", p=P, d=C)[:, dt, :],
        )
        gb_t = gb_pool.tile([P, C], F32R, name=f"gb{dt}", tag="gb")
        nc.vector.tensor_copy(gb_t, gb_stage)     # rounding copy
        for j in range(CT):
            nc.tensor.matmul(
                V_ps[j],
                gb_t[:, j * P:(j + 1) * P],
                WkT[:, dt, :],
                start=(dt == 0),
                stop=(dt == CT - 1),
            )
    V_sb = chain.tile([P, CT, C], F32R, name="V_sb", tag="c8")
    for j in range(CT):
        nc.scalar.copy(V_sb[:, j, :], V_ps[j])

    # ---------------- S = WqT_a^T @ V + rank1 ; softmax ----------------
    attn = chain.tile([P, CT, C], F32R, name="attn", tag="c8")
    S_ps = [ps_g.tile([P, C], F32, name=f"S_ps{j}", tag="g") for j in range(CT)]
    for j in range(CT):
        for ct in range(CT):
            nc.tensor.matmul(
                S_ps[j],
                WqT[:, ct, j * P:(j + 1) * P],
                V_sb[:, ct, :],
                start=(ct == 0),
                stop=False,
            )
        nc.tensor.matmul(
            S_ps[j], Lpad[:, j, :], Rpad, start=False, stop=True
        )
        mx = small.tile([P, 1], F32, name=f"mx{j}")
        nc.vector.reduce_max(mx, S_ps[j], axis=AX.X)
        mb = small.tile([P, 1], F32, name=f"mb{j}")
        nc.vector.tensor_scalar_mul(mb, mx, -SCALE)
        rs = small.tile([P, 1], F32, name=f"rs{j}")
        nc.scalar.activation(
            attn[:, j, :], S_ps[j], ACTF.Exp,
            bias=mb, scale=SCALE, accum_out=rs,
        )
        rrec = small.tile([P, 1], F32, name=f"rrec{j}")
        nc.vector.reciprocal(rrec, rs)
        nc.vector.tensor_scalar_mul(attn[:, j, :], f32_(attn[:, j, :]), rrec)

    # ---------------- attnT ----------------
    attnT = chain.tile([P, CT, C], F32R, name="attnT", tag="c8")
    for ct in range(CT):
        at_ps = ps_t.tile([P, C], F32, name=f"at_ps{ct}", tag="pt")
        for j in range(CT):
            nc.tensor.transpose(
                r_(at_ps[:, j * P:(j + 1) * P]),
                attn[:, j, ct * P:(ct + 1) * P],
                ident_r,
            )
        nc.scalar.copy(attnT[:, ct, :], at_ps)

    # ---------------- M0 = attn @ Wv ----------------
    Wv_r = chain.tile([P, CT, C], F32R, name="Wv_r", tag="c8")
    for ot in range(CT):
        wv_stage = gb_pool.tile([P, C], F32, name=f"wvs{ot}", tag="gb")
        nc.sync.dma_start(
            wv_stage,
            qkv_w_ext[2 * C:3 * C, :].rearrange("(j p) c -> p j c", p=P)[:, ot, :],
        )
        nc.vector.tensor_copy(Wv_r[:, ot, :], wv_stage)  # cast -> f32r

    M0 = chain.tile([P, CT, C], F32R, name="M0", tag="c8")
    M0_ps = [ps_g.tile([P, C], F32, name=f"M0_ps{j}", tag="g") for j in range(CT)]
    for j in range(CT):
        for ot in range(CT):
            nc.tensor.matmul(
                M0_ps[j],
                attnT[:, ot, j * P:(j + 1) * P],
                Wv_r[:, ot, :],
                start=(ot == 0),
                stop=(ot == CT - 1),
            )
        nc.scalar.copy(M0[:, j, :], M0_ps[j])

    # w3 = attn @ bv
    w3 = small.tile([P, CT], F32, name="w3")
    for j in range(CT):
        w3_ps = ps_t.tile([P, 1], F32, name=f"w3_ps{j}", tag="pt")
        for ot in range(CT):
            nc.tensor.matmul(
                w3_ps,
                f32_(attnT[:, ot, j * P:(j + 1) * P]),
                qkvb_sb[:, 2 * CT + ot:2 * CT + ot + 1],
                start=(ot == 0),
                stop=(ot == CT - 1),
            )
        nc.vector.tensor_copy(w3[:, j:j + 1], w3_ps)

    # ---------------- A0 = Pw @ M0 ----------------
    A0 = chain.tile([P, CT, C], F32R, name="A0", tag="c8")
    A0_ps = [ps_g.tile([P, C], F32, name=f"A0_ps{j}", tag="g") for j in range(CT)]
    for j in range(CT):
        for mt in range(CT):
            nc.tensor.matmul(
                A0_ps[j],
                PwT[:, mt, j * P:(j + 1) * P],
                M0[:, mt, :],
                start=(mt == 0),
                stop=(mt == CT - 1),
            )
        nc.scalar.copy(A0[:, j, :], A0_ps[j])

    # pw3 = Pw @ w3
    pw3 = small.tile([P, CT], F32, name="pw3")
    for j in range(CT):
        pw3_ps = ps_t.tile([P, 1], F32, name=f"pw3_ps{j}", tag="pt")
        for mt in range(CT):
            nc.tensor.matmul(
                pw3_ps,
                f32_(PwT[:, mt, j * P:(j + 1) * P]),
                w3[:, mt:mt + 1],
                start=(mt == 0),
                stop=(mt == CT - 1),
            )
        nc.vector.tensor_copy(pw3[:, j:j + 1], pw3_ps)

    # ---------------- A0T; ab = A0 @ bvec; delta; scale ----------------
    A0T = chain.tile([P, CT, C], F32R, name="A0T", tag="c8")
    for ct in range(CT):
        a0t_ps = ps_t.tile([P, C], F32, name=f"a0t_ps{ct}", tag="pt")
        for j in range(CT):
            nc.tensor.transpose(
                r_(a0t_ps[:, j * P:(j + 1) * P]),
                A0[:, j, ct * P:(ct + 1) * P],
                ident_r,
            )
        nc.scalar.copy(A0T[:, ct, :], a0t_ps)

    ab = small.tile([P, CT], F32, name="ab")
    for j in range(CT):
        ab_ps = ps_t.tile([P, 1], F32, name=f"ab_ps{j}", tag="pt")
        for ct in range(CT):
            nc.tensor.matmul(
                ab_ps,
                f32_(A0T[:, ct, j * P:(j + 1) * P]),
                bvec[:, ct:ct + 1],
                start=(ct == 0),
                stop=(ct == CT - 1),
            )
        nc.vector.tensor_copy(ab[:, j:j + 1], ab_ps)

    delta = small.tile([P, CT], F32, name="delta")
    nc.vector.tensor_add(out=delta, in0=ab, in1=pw3)
    nc.vector.tensor_add(out=delta, in0=delta, in1=pb_sb)

    # lhsT_A = diag(a) @ A0T  (row scale, in place)
    for ct in range(CT):
        nc.vector.tensor_scalar_mul(
            A0T[:, ct, :], f32_(A0T[:, ct, :]), a_sb[:, ct:ct + 1]
        )

    # ---------------- phase 2: out = x + A x + delta ----------------
    for u in range(PH2_ITERS):
        sl = slice(u * PH2_CHUNK, (u + 1) * PH2_CHUNK)
        for j in range(CT):
            y_ps = ps_y.tile([P, PH2_CHUNK], F32, name=f"y_ps{u}_{j}", tag="y")
            for ct in range(CT):
                nc.tensor.matmul(
                    y_ps,
                    A0T[:, ct, j * P:(j + 1) * P],
                    x_res[:, ct, sl],
                    start=(ct == 0),
                    stop=(ct == CT - 1),
                )
            y_sb = y_pool.tile([P, PH2_CHUNK], F32, name=f"y_sb{u}_{j}", tag="y")
            nc.vector.scalar_tensor_tensor(
                out=y_sb,
                in0=y_ps,
                scalar=delta[:, j:j + 1],
                in1=f32_(x_res[:, j, sl]),
                op0=OP.add,
                op1=OP.add,
            )
            nc.sync.dma_start(out_view[:, j, sl], y_sb)

    ctx.close()


_CACHED_NC = None


def _get_nc():
    global _CACHED_NC
    if _CACHED_NC is None:
        _CACHED_NC = build_graph()
    return _CACHED_NC


def make_in_maps(inputs):
    xf = np.ascontiguousarray(
        np.asarray(inputs["x"], dtype=np.float32).reshape(B, C, N)
    )
    rep = {
        k: np.ascontiguousarray(np.asarray(inputs[k], dtype=np.float32))
        for k in ("gn_w", "gn_b", "qkv_w", "qkv_b", "proj_w", "proj_b")
    }
    ii = np.arange(P) // GSIZE
    rep["adjc"] = np.ascontiguousarray(
        (ii[:, None] == ii[None, :]).astype(np.float32)
    )
    in_maps = []
    for i in range(NCORES):
        b, sh = divmod(i, SHARDS)
        m = {"x": np.ascontiguousarray(xf[b, :, sh * NS:(sh + 1) * NS])}
        m.update(rep)
        in_maps.append(m)
    return in_maps


def assemble(results, inputs):
    x = np.asarray(inputs["x"])
    out = np.empty((B, C, N), dtype=np.float32)
    for i in range(NCORES):
        b, sh = divmod(i, SHARDS)
        out[b, :, sh * NS:(sh + 1) * NS] = results[i]["out"]
    return out.reshape(x.shape)


def kernel(**inputs) -> np.ndarray:
    nc = _get_nc()
    res = run_bass_kernel_spmd(nc, make_in_maps(inputs), list(range(NCORES)))
    return assemble(res.results, inputs)


if __name__ == "__main__":
    # quick smoke: build only
    build_graph()
    print("build OK")
